# revision 29
# baseline (speedup 1.0000x reference)
"""TRN2 Bass kernel for nn_BlockPermProduct (measured 184-186 us at the warm
2.4 GHz clock vs the 205 us prior baseline; rel err 4.2e-3 vs the 2e-2 gate).

The reference applies 9 probabilistic block-permutation mixing steps to each
row of x [65536, 1024]. Every step is linear in x, so the whole transform is
``out = x @ M^T`` for a 1024x1024 matrix M depending only on the (9, 3)
logits; M is built on the host in float64 by pushing the identity through the
reference transform.

1. Exact block sparsity. Under the feature grouping g = b0 + 2*b1 + 4*b9
   (bits of the feature index), M has exact zero blocks: first-half outputs
   (b9=0) never depend on inputs with (b9=1 & b0=1); second-half outputs
   never depend on inputs with (b9=0 & b0=0). Each 512-row block needs
   2 halves x 4 out-blocks x 6 accumulating 512-wide bf16 matmuls (48 =
   0.75x dense; matmul moving width is capped at 512 by the PSUM bank).
   48 x 16 x 216ns ~ 166us is the PE instruction-rate floor for this
   support structure: every 128-output coset depends on the same 768
   inputs under ANY bit grouping (verified numerically for all 3-bit and
   4-bit groupings and all factor splits), so >= 6 contract tiles per out
   tile; fp8 fails the 2e-2 gate (x-only e4m3 is 3.3e-2; the hybrid
   smallest-two-blocks-in-fp8 DoubleRow variant sims at 1.9e-2 - no
   margin); vector engines are ~40x too slow for the factored form.

2. ALL layout gathers happen on the host (pure reshape/transpose - the
   bit-grouping factors exactly): x is packed per core to
   xtp[f, step, g, r] so every device load is ONE flat full-rate
   [128, 4096] DMA per step; the output is produced in the same packed
   layout and un-packed on the host. bf16 I/O halves both DMA directions.
   Strided gather DMAs measured only ~200 GB/s and serialized the head;
   flat runs at ~341 GB/s.

3. Edge scheduling (perfetto/NTFF-driven):
   - PE warmup burst: 42 junk 128-wide matmuls at t~0 keep the PE busy
     through the HAM activity window so the clock gate is open (2.4 GHz
     not 1.2) when the first real matmul issues. Fine granularity ends
     the burst just before data-ready, so the real matmul never waits
     on a draining warmup matmul (idle > ~3.4us would re-throttle; a
     sub-us idle seam is free).
   - Strict serial head priority on ONE HWDGE ring (cross-ring DMAs
     share HBM bandwidth, so a second ring only delays critical bytes):
     x groups 0-6, M q0 stationaries, M q1, x group 7, M q23, x step 1.
     First real matmul at ~13us, warm, zero stalls afterwards.
   - One flat load + one flat store DMA per step; 2-step load lookahead.
   - One PSUM bank per (half, quarter) accumulation chain (8 tags) with
     per-quarter casts: DVE for half 0, ACT for half 1 (the measured-best
     engine split; flat contiguous APs only - strided cast APs cost
     ~43ns on every matmul).
   - The final step stores in 4 pieces so the teardown drain is gated on
     a 128 KiB transfer, not 1 MiB.

Remaining known overheads (framework/measurement-internal, ~10us total):
a ~432ns stall wherever the NTFF trace capture starts a new segment
(~once per step; the delayed matmul carries trace=START), and the
end-of-context semaphore zeroing cascade (~50 EVENT_SEMAPHORE
instructions per engine after the final barrier).

Sharding: pure data parallel over the batch dim across 8 cores (SPMD, no
communication); M is replicated.
"""

import numpy as np
from contextlib import ExitStack

import ml_dtypes

import concourse.bass as bass
import concourse.bacc as bacc
import concourse.mybir as mybir
import concourse.tile as tile
from concourse.bass_utils import run_bass_kernel_spmd

BATCH = 65536
SIZE = 1024
N_CORES = 8
ROWS_PER_CORE = BATCH // N_CORES  # 8192
P = 128
RW = 512
N_STEPS = ROWS_PER_CORE // RW  # 16

F32 = mybir.dt.float32
BF16 = mybir.dt.bfloat16
NP_BF16 = ml_dtypes.bfloat16

KO_HALF0 = [0, 1, 2, 3, 4, 6]
KO_HALF1 = [1, 3, 4, 5, 6, 7]

TRACE = False
TRACE_KWARGS = {}
LAST_RESULTS = None

_NC_CACHE = {}


def _transform64(y, logits):
    m = 10
    sizes = [SIZE >> i for i in range(m - 1)][::-1]
    out = y
    for i in range(m - 2, -1, -1):
        n = sizes[i]
        p = 1.0 / (1.0 + np.exp(-logits[i].astype(np.float64)))
        z = out.reshape(-1, n)
        sep = z.reshape(-1, n // 2, 2).transpose(0, 2, 1).reshape(-1, n)
        z = (1 - p[0]) * z + p[0] * sep
        h = n // 2
        first = (1 - p[1]) * z[:, :h] + p[1] * z[:, h - 1::-1]
        second = (1 - p[2]) * z[:, h:] + p[2] * z[:, : h - 1 : -1]
        out = np.concatenate([first, second], axis=1).reshape(out.shape)
    return out


def _build_m(logits):
    eye = np.eye(SIZE, dtype=np.float64)
    mt = _transform64(eye, logits)
    return mt.T


def _feat(g, f):
    return 512 * (g >> 2) + 4 * f + (g & 3)


_GROUP_FEATS = [np.array([_feat(g, f) for f in range(P)]) for g in range(8)]


def _check_sparse(m):
    for o in range(8):
        rows = _GROUP_FEATS[o]
        banned = [5, 7] if o < 4 else [0, 2]
        for i in banned:
            cols = _GROUP_FEATS[i]
            if np.abs(m[np.ix_(rows, cols)]).max() > 1e-12:
                return False
    return True


def _ko_half(sparse):
    return [KO_HALF0, KO_HALF1] if sparse else [list(range(8))] * 2


def _build_mtp(m, sparse):
    """Packed stationaries [128, 4*nblk*128], q-major: block (q, h, idx)
    holds the (in-group i=ko_half[h][idx] -> out-group 4h+q) stationary."""
    ko_half = _ko_half(sparse)
    nko = len(ko_half[0])
    nblk = 2 * nko
    mtp = np.zeros((P, 4 * nblk * P), dtype=np.float64)
    for q in range(4):
        for h in range(2):
            for idx, i in enumerate(ko_half[h]):
                o = 4 * h + q
                k = q * nblk + nko * h + idx
                mtp[:, k * P : (k + 1) * P] = m[
                    np.ix_(_GROUP_FEATS[o], _GROUP_FEATS[i])
                ].T
    return np.ascontiguousarray(mtp.astype(NP_BF16))


def _pack_x(xb_core):
    """[8192, 1024] bf16 -> [128, N_STEPS*4096] so each step's tile is one
    flat slice with free layout (g=4h+q, r)."""
    t = xb_core.reshape(N_STEPS, RW, 2, P, 4)  # [s, r, h, f, q]
    t = t.transpose(3, 0, 2, 4, 1)  # [f, s, h, q, r]
    return np.ascontiguousarray(t).reshape(P, N_STEPS * 8 * RW)


def _unpack_out(op_core):
    """Inverse of _pack_x for the output: [128, N_STEPS*4096] -> [8192, 1024]."""
    t = op_core.reshape(P, N_STEPS, 2, 4, RW)  # [f, s, h, q, r]
    t = t.transpose(1, 4, 2, 0, 3)  # [s, r, h, f, q]
    return np.ascontiguousarray(t).reshape(ROWS_PER_CORE, SIZE)


def _build_bass(sparse):
    ko_half = _ko_half(sparse)
    nko = len(ko_half[0])  # 6 sparse, 8 dense
    nblk = 2 * nko
    nc = bacc.Bacc("TRN2", target_bir_lowering=False, debug=False)
    xtp = nc.dram_tensor(
        "xtp", [P, N_STEPS * 8 * RW], BF16, kind="ExternalInput"
    ).ap()
    mtg = nc.dram_tensor("mtp", [P, 4 * nblk * P], BF16, kind="ExternalInput").ap()
    outp = nc.dram_tensor(
        "outp", [P, N_STEPS * 8 * RW], BF16, kind="ExternalOutput"
    ).ap()

    with tile.TileContext(nc) as tc, ExitStack() as ctx:
        const = ctx.enter_context(tc.tile_pool(name="const", bufs=1))
        xpool = ctx.enter_context(tc.tile_pool(name="xin", bufs=3))
        opool = ctx.enter_context(tc.tile_pool(name="osb", bufs=2))
        pso = ctx.enter_context(tc.tile_pool(name="pso", bufs=1, space="PSUM"))

        # --- PE warmup: junk matmuls to flip the HAM clock gate to 8/8
        # while the first loads are in flight. Output goes to half-0's
        # PSUM tile (tag po0); step 0's first accumulation chain WAR-waits
        # on these, which is harmless (warmup ends ~11.5us, the first real
        # matmul's inputs land ~11us).
        warm = const.tile([P, RW], BF16, tag="warm")
        nc.gpsimd.memset(warm[:], 0.0)
        # Fine-grained (N=128) junk matmuls: the burst ends just BEFORE
        # the first real matmul's data lands, so the real matmul never
        # waits on an in-flight 427ns warmup matmul draining (PE idle
        # before data-ready is free; warmup overshoot is not).
        wq = pso.tile([P, RW], F32, tag="po00")
        for _ in range(42):
            nc.tensor.matmul(wq[:, 0:P], warm[:, 0:P], warm[:, 0:P], start=True, stop=True)

        # --- tiles
        mtp = const.tile([P, 4 * nblk * P], BF16, tag="mtp")

        def stat(h, idx, q):
            k = q * nblk + nko * h + idx
            return mtp[:, k * P : (k + 1) * P]

        def load_x(step):
            t = xpool.tile([P, 8 * RW], BF16, tag="xin")
            nc.sync.dma_start(
                t[:], xtp[:, step * 8 * RW : (step + 1) * 8 * RW]
            )
            return t

        # --- head: everything on the sync ring in strict priority order
        # (same-ring DMAs transfer serially at full rate; cross-ring ones
        # share HBM bandwidth and delay the critical bytes). First real
        # matmul needs x0 + M-q01; q23 stationaries are needed ~2.6us
        # later; x1 a full step later.
        # Head priority: the first accumulation chain (h0,q0) needs x
        # groups 0-6 and the q0 stationaries; q1 stationaries ~1.3us
        # later; group 7 by the h1 phase; q23 by mid-step; x1 a full
        # step later. Strict serial order on one ring = strict priority.
        xin_tiles = [None] * N_STEPS
        x0 = xpool.tile([P, 8 * RW], BF16, tag="xin")
        nc.sync.dma_start(x0[:, : 7 * RW], xtp[:, : 7 * RW])
        nc.sync.dma_start(mtp[:, : nblk * P], mtg[:, : nblk * P])
        nc.sync.dma_start(mtp[:, nblk * P : 2 * nblk * P], mtg[:, nblk * P : 2 * nblk * P])
        nc.sync.dma_start(x0[:, 7 * RW :], xtp[:, 7 * RW : 8 * RW])
        nc.sync.dma_start(mtp[:, 2 * nblk * P :], mtg[:, 2 * nblk * P :])
        xin_tiles[0] = x0
        xin_tiles[1] = load_x(1)

        for step in range(N_STEPS):
            if step + 2 < N_STEPS:
                xin_tiles[step + 2] = load_x(step + 2)
            xin = xin_tiles[step]
            xin_tiles[step] = None
            osb = opool.tile([P, 8 * RW], BF16, tag="osb")

            for h in range(2):
                ko = ko_half[h]
                for q in range(4):
                    # One PSUM bank per (h, q): finest-grain bank
                    # recycling, so a chain start only WARs on the cast
                    # of the same quarter two half-phases back.
                    po = pso.tile([P, RW], F32, tag=f"po{h}{q}")
                    for idx, i in enumerate(ko):
                        nc.tensor.matmul(
                            po[:],
                            stat(h, idx, q),
                            xin[:, i * RW : (i + 1) * RW],
                            start=(idx == 0),
                            stop=(idx == len(ko) - 1),
                        )
                    # PSUM->SBUF casts: DVE half 0, ACT half 1
                    # (measured-best split); flat contiguous APs.
                    eng = nc.vector.tensor_copy if h == 0 else nc.scalar.copy
                    g = 4 * h + q
                    eng(osb[:, g * RW : (g + 1) * RW], po[:])

            s0 = step * 8 * RW
            if step == N_STEPS - 1:
                # Drain the tail in pieces: the run's critical path ends at
                # the last store's completion receipt, so keep it small.
                nc.sync.dma_start(outp[:, s0 : s0 + 4 * RW], osb[:, : 4 * RW])
                nc.sync.dma_start(
                    outp[:, s0 + 4 * RW : s0 + 6 * RW], osb[:, 4 * RW : 6 * RW]
                )
                nc.sync.dma_start(
                    outp[:, s0 + 6 * RW : s0 + 7 * RW], osb[:, 6 * RW : 7 * RW]
                )
                nc.sync.dma_start(
                    outp[:, s0 + 7 * RW : s0 + 8 * RW], osb[:, 7 * RW : 8 * RW]
                )
            else:
                nc.scalar.dma_start(outp[:, s0 : s0 + 8 * RW], osb[:])

    nc.compile()
    return nc


def _get_nc(sparse):
    key = bool(sparse)
    if key not in _NC_CACHE:
        _NC_CACHE[key] = _build_bass(key)
    return _NC_CACHE[key]


def kernel(x, logits):
    x = np.asarray(x)
    logits = np.asarray(logits)
    assert x.shape == (BATCH, SIZE)

    m = _build_m(logits)
    sparse = _check_sparse(m)
    mtp = _build_mtp(m, sparse)
    nc = _get_nc(sparse)

    xb = x.astype(NP_BF16)
    in_maps = [
        {
            "xtp": _pack_x(xb[i * ROWS_PER_CORE : (i + 1) * ROWS_PER_CORE]),
            "mtp": mtp,
        }
        for i in range(N_CORES)
    ]
    kwargs = dict(TRACE_KWARGS)
    if TRACE:
        kwargs.setdefault("trace", True)
        kwargs.setdefault("trace_cores", [0])
    res = run_bass_kernel_spmd(nc, in_maps, core_ids=list(range(N_CORES)), **kwargs)
    global LAST_RESULTS
    LAST_RESULTS = res
    outs = [
        _unpack_out(np.asarray(res.results[i]["outp"])) for i in range(N_CORES)
    ]
    return np.ascontiguousarray(np.concatenate(outs, axis=0)).astype(np.float32)
